# revision 5
# baseline (speedup 1.0000x reference)
"""Trainium2 Bass kernel for nn_MessagePassing (gnn_message_passing).

Math (per batch b):
    coef[s,e] = sum_o adj[s,o] * edge[s,o,e]
    v[s,e,i]  = sum_j W[e,i,j] * node[s,j]
    out[s,i]  = sum_e coef[s,e] * v[s,e,i]

Sharding: data parallel over the batch axis — core b handles batch b.

Pipeline design (per core, ~37MB HBM read => ~105us DMA floor):
  * edge (32MB f32) is DMA'd with an f32->bf16 cast on the SWDGE (gpsimd)
    queue; all 8 s-tiles stay resident in SBUF (16KB/partition each) so the
    DMA stream is never backpressured by compute.
  * adj stays f32, loaded per s-tile chunk on the sync HWDGE queue (so the
    first coef op can start ~3us in); node/W ride the scalar HWDGE queue.
  * coef: per (s-tile, e) one DVE scalar_tensor_tensor with a strided bf16
    in0 read (STT has no 2x uops, so de-interleaving buys little; bf16
    halves the stride penalty vs f32).  Optional K_DEINT path offloads
    the first K e's to ScalarE (de-interleave copy) + DVE tensor_tensor
    (bf16 2x) + ScalarE accum-reduce, to rebalance DVE <-> ScalarE.
  * v: PE matmuls, f32 (nodeT stationary, W^T moving), independent of coef.
  * out: 8 ScalarE activation muls (scale = coef[:,e] per-partition) into
    an SBUF slab, then one DVE reduce over e.
  * s-tiles 0 and 7 are processed in o-halves to shrink pipeline ramp/tail.
"""

import numpy as np
from contextlib import ExitStack

import concourse.bass as bass
import concourse.bacc as bacc
import concourse.mybir as mybir
import concourse.tile as tile
from concourse.bass_utils import run_bass_kernel_spmd
from concourse.masks import make_identity

B, N, D, E = 8, 1024, 128, 8
P = 128
NT = N // P  # 8 s-tiles per core
H = N // 2

F32 = mybir.dt.float32
BF16 = mybir.dt.bfloat16
MUL = mybir.AluOpType.mult
ADD = mybir.AluOpType.add

K_DEINT = 0  # e's in [0, K_DEINT) take the ScalarE-deinterleave + DVE-TT path
SPLIT_TILES = (0, NT - 1)  # s-tiles processed in o-halves


def build_nc():
    nc = bacc.Bacc("TRN2", target_bir_lowering=False, debug=False, num_devices=B)

    node_d = nc.dram_tensor("node_state", [N, D], F32, kind="ExternalInput").ap()
    edge_d = nc.dram_tensor("edge_type_mat", [N, N, E], F32, kind="ExternalInput").ap()
    adj_d = nc.dram_tensor("adj_mat", [N, N], F32, kind="ExternalInput").ap()
    w_d = nc.dram_tensor("W", [E, D, D], F32, kind="ExternalInput").ap()
    out_d = nc.dram_tensor("out", [N, D], F32, kind="ExternalOutput").ap()

    with tile.TileContext(nc) as tc, ExitStack() as ctx:
        const_pool = ctx.enter_context(tc.tile_pool(name="const", bufs=1))
        edge_pool = ctx.enter_context(tc.tile_pool(name="edge", bufs=1))
        edgeh_pool = ctx.enter_context(tc.tile_pool(name="edgeh", bufs=1))
        work_pool = ctx.enter_context(tc.tile_pool(name="work", bufs=2))
        slab_pool = ctx.enter_context(tc.tile_pool(name="slab", bufs=2))
        psum_pool = ctx.enter_context(tc.tile_pool(name="psum", bufs=8, space="PSUM"))

        ident = const_pool.tile([P, P], F32)
        make_identity(nc, ident[:])

        # --- edge stream: SWDGE cast f32 -> bf16, all tiles resident -------
        edge_full = {}   # t -> [P, N, E] bf16
        edge_half = {}   # (t, h) -> [P, H, E] bf16
        for t in range(NT):
            if t in SPLIT_TILES:
                for h in range(2):
                    eh = edgeh_pool.tile([P, H, E], BF16, tag=f"edgeh{t}{h}")
                    nc.gpsimd.dma_start(
                        eh[:], edge_d[bass.ts(t, P), bass.ts(h, H)]
                    )
                    edge_half[(t, h)] = eh
            else:
                et = edge_pool.tile([P, N, E], BF16, tag=f"edge{t}")
                nc.gpsimd.dma_start(et[:], edge_d[bass.ts(t, P)])
                edge_full[t] = et

        # --- resident f32 inputs ------------------------------------------
        # adj in per-tile chunks on sync HWDGE so tile 0 is ready early.
        adj_all = const_pool.tile([P, NT, N], F32)
        for t in range(NT):
            nc.sync.dma_start(adj_all[:, t, :], adj_d[bass.ts(t, P)])
        node_all = const_pool.tile([P, NT, D], F32)
        nc.scalar.dma_start(node_all[:], node_d.rearrange("(t p) j -> p t j", p=P))
        w_all = const_pool.tile([P, E, D], F32)  # [i, e, j]
        nc.scalar.dma_start(w_all[:], w_d.rearrange("e i j -> i e j"))

        # node^T[j, s] and W[e]^T[j, i] via PE transpose.
        nodeT = const_pool.tile([P, N], F32)
        for t in range(NT):
            pt = psum_pool.tile([P, P], F32, tag="psum")
            nc.tensor.transpose(pt[:], node_all[:, t, :], ident[:])
            nc.scalar.copy(nodeT[:, bass.ts(t, P)], pt[:])
        wT = const_pool.tile([P, E, D], F32)  # [j, e, i]
        for e in range(E):
            pt = psum_pool.tile([P, P], F32, tag="psum")
            nc.tensor.transpose(pt[:], w_all[:, e, :], ident[:])
            nc.scalar.copy(wT[:, e, :], pt[:])

        scratch = const_pool.tile([P, N], BF16)  # STT mandatory product out

        def coef_stt(in0, adj_ap, accum):
            # accum[:] = sum_o in0[:, o(, e)] * adj[:, o]
            nc.vector.scalar_tensor_tensor(
                out=scratch[:, : adj_ap.shape[-1]],
                in0=in0,
                scalar=1.0,
                in1=adj_ap,
                op0=MUL,
                op1=MUL,
                accum_out=accum,
            )

        for t in range(NT):
            coef = work_pool.tile([P, E], F32, tag="coef")
            if t in SPLIT_TILES:
                coefh = work_pool.tile([P, 2, E], F32, tag="coefh")
                for h in range(2):
                    eh = edge_half[(t, h)]
                    adj_ap = adj_all[:, t, bass.ts(h, H)]
                    for e in range(E):
                        coef_stt(eh[:, :, e], adj_ap, coefh[:, h, e : e + 1])
                nc.vector.tensor_tensor(
                    out=coef[:], in0=coefh[:, 0, :], in1=coefh[:, 1, :], op=ADD
                )
            else:
                et = edge_full[t]
                for e in range(E):
                    coef_stt(et[:, :, e], adj_all[:, t, :], coef[:, e : e + 1])

            # V[s, e, i] for 4 e's per matmul (512-col moving operand), f32.
            psums = []
            for g in range(E // 4):
                pv = psum_pool.tile([P, 4, D], F32, tag="psum")
                nc.tensor.matmul(
                    pv[:],
                    lhsT=nodeT[:, bass.ts(t, P)],
                    rhs=wT[:, g * 4 : (g + 1) * 4, :],
                    start=True,
                    stop=True,
                )
                psums.append(pv)

            # out[s, i] = sum_e coef[s, e] * v[s, e, i]
            slab = slab_pool.tile([P, E, D], F32, tag="slab")
            for e in range(E):
                nc.scalar.mul(
                    slab[:, e, :], psums[e // 4][:, e % 4, :], coef[:, e : e + 1]
                )
            out_sb = work_pool.tile([P, D], F32, tag="out_sb")
            nc.vector.reduce_sum(
                out_sb[:], slab[:].rearrange("p e i -> p i e"),
                axis=mybir.AxisListType.X,
            )
            nc.sync.dma_start(out_d[bass.ts(t, P)], out_sb[:])

    nc.compile()
    return nc


_NC_CACHE = None


def get_nc():
    global _NC_CACHE
    if _NC_CACHE is None:
        _NC_CACHE = build_nc()
    return _NC_CACHE


def make_in_maps(node_state, edge_type_mat, adj_mat, W):
    return [
        {
            "node_state": np.ascontiguousarray(node_state[b], dtype=np.float32),
            "edge_type_mat": np.ascontiguousarray(edge_type_mat[b], dtype=np.float32),
            "adj_mat": np.ascontiguousarray(adj_mat[b], dtype=np.float32),
            "W": np.ascontiguousarray(W, dtype=np.float32),
        }
        for b in range(B)
    ]


def kernel(node_state, edge_type_mat, adj_mat, W):
    nc = get_nc()
    in_maps = make_in_maps(node_state, edge_type_mat, adj_mat, W)
    res = run_bass_kernel_spmd(nc, in_maps, list(range(B)))
    return np.stack([res.results[b]["out"] for b in range(B)], axis=0)


# revision 6
# speedup vs baseline: 1.4974x; 1.4974x over previous
"""Trainium2 Bass kernel for nn_MessagePassing (gnn_message_passing).

Math (per batch b):
    coef[s,e] = sum_o adj[s,o] * edge[s,o,e]
    v[s,e,i]  = sum_j W[e,i,j] * node[s,j]
    out[s,i]  = sum_e coef[s,e] * v[s,e,i]

Sharding: data parallel over the batch axis — core b handles batch b.

Design notes (per core, ~36.5MB HBM read => ~103us DMA floor @358GB/s):
  * edge is transposed ON THE HOST to [E, N, N] so each e-stream is
    unit-stride on chip (strided SBUF reads cost ~2ns/elem on every
    engine — measured — which made the [s,o,e] layout uncompetitive).
  * edge is DMA'd with an f32->bf16 cast on the SWDGE (gpsimd) queue;
    all 8 s-tiles stay resident in SBUF (16KB/partition each) so the
    stream is never backpressured.  s-tiles 0/7 are split into o-halves
    to shrink pipeline ramp/tail.
  * adj: f32 per-tile chunks on the sync HWDGE queue (concurrent with
    the SWDGE stream), cast to bf16 by ScalarE on chip.
  * coef: per (s-tile, e) a DVE tensor_tensor bf16 multiply (2x_1p mode,
    ~0.69us) + a ScalarE activation-accumulate reduce (~0.4us).
    (scalar_tensor_tensor / tensor_tensor_reduce have no 2x uops.)
  * v: PE matmuls, f32 (nodeT stationary, W^T moving), independent of coef.
  * out: 8 ScalarE activation muls (scale = coef[:,e] per-partition) into
    a [P, i, e] slab, then one unit-stride DVE reduce over e.
"""

import numpy as np
from contextlib import ExitStack

import concourse.bass as bass
import concourse.bacc as bacc
import concourse.mybir as mybir
import concourse.tile as tile
from concourse.bass_utils import run_bass_kernel_spmd
from concourse.masks import make_identity

B, N, D, E = 8, 1024, 128, 8
P = 128
NT = N // P  # 8 s-tiles per core
H = N // 2

F32 = mybir.dt.float32
BF16 = mybir.dt.bfloat16
MUL = mybir.AluOpType.mult
ADD = mybir.AluOpType.add
COPY = mybir.ActivationFunctionType.Copy

SPLIT_TILES = (0, NT - 1)  # s-tiles processed in o-halves


def build_nc():
    nc = bacc.Bacc("TRN2", target_bir_lowering=False, debug=False, num_devices=B)

    node_d = nc.dram_tensor("node_state", [N, D], F32, kind="ExternalInput").ap()
    # transposed on the host: [E, N(s), N(o)]
    edge_d = nc.dram_tensor("edge_type_mat", [E, N, N], F32, kind="ExternalInput").ap()
    adj_d = nc.dram_tensor("adj_mat", [N, N], F32, kind="ExternalInput").ap()
    w_d = nc.dram_tensor("W", [E, D, D], F32, kind="ExternalInput").ap()
    out_d = nc.dram_tensor("out", [N, D], F32, kind="ExternalOutput").ap()

    with tile.TileContext(nc) as tc, ExitStack() as ctx:
        const_pool = ctx.enter_context(tc.tile_pool(name="const", bufs=1))
        edge_pool = ctx.enter_context(tc.tile_pool(name="edge", bufs=1))
        adjf_pool = ctx.enter_context(tc.tile_pool(name="adjf", bufs=2))
        work_pool = ctx.enter_context(tc.tile_pool(name="work", bufs=2))
        prod_pool = ctx.enter_context(tc.tile_pool(name="prod", bufs=3))
        slab_pool = ctx.enter_context(tc.tile_pool(name="slab", bufs=2))
        psum_pool = ctx.enter_context(tc.tile_pool(name="psum", bufs=8, space="PSUM"))

        ident = const_pool.tile([P, P], F32)
        make_identity(nc, ident[:])

        # --- edge stream: SWDGE cast f32 -> bf16, all tiles resident -------
        # dest [p, e, o]; src per partition-line: E blocks of N f32.
        edge_src = edge_d.rearrange("e (t p) o -> p t e o", p=P)
        edge_tiles = {}  # t -> ([P, E, N] bf16, or per-half [P, E, H])
        for t in range(NT):
            if t in SPLIT_TILES:
                halves = []
                for h in range(2):
                    eh = edge_pool.tile([P, E, H], BF16, tag=f"edgeh{t}{h}")
                    nc.gpsimd.dma_start(eh[:], edge_src[:, t, :, bass.ts(h, H)])
                    halves.append(eh)
                edge_tiles[t] = halves
            else:
                et = edge_pool.tile([P, E, N], BF16, tag=f"edge{t}")
                nc.gpsimd.dma_start(et[:], edge_src[:, t, :, :])
                edge_tiles[t] = et

        # --- adj: f32 chunks on sync HWDGE, cast to bf16 on ScalarE --------
        adj_bf = const_pool.tile([P, NT, N], BF16)
        adj_f32 = {}
        for t in range(NT):
            af = adjf_pool.tile([P, N], F32, tag="adjf")
            nc.sync.dma_start(af[:], adj_d[bass.ts(t, P)])
            adj_f32[t] = af

        node_all = const_pool.tile([P, NT, D], F32)
        nc.scalar.dma_start(node_all[:], node_d.rearrange("(t p) j -> p t j", p=P))
        w_all = const_pool.tile([P, E, D], F32)  # [i, e, j]
        nc.scalar.dma_start(w_all[:], w_d.rearrange("e i j -> i e j"))

        # node^T[j, s] and W[e]^T[j, i] via PE transpose (copies on DVE,
        # which is idle until the first edge tile lands).
        nodeT = const_pool.tile([P, N], F32)
        for t in range(NT):
            pt = psum_pool.tile([P, P], F32, tag="psum")
            nc.tensor.transpose(pt[:], node_all[:, t, :], ident[:])
            nc.vector.tensor_copy(nodeT[:, bass.ts(t, P)], pt[:])
        wT = const_pool.tile([P, E, D], F32)  # [j, e, i]
        for e in range(E):
            pt = psum_pool.tile([P, P], F32, tag="psum")
            nc.tensor.transpose(pt[:], w_all[:, e, :], ident[:])
            nc.vector.tensor_copy(wT[:, e, :], pt[:])

        scratch = const_pool.tile([P, N], BF16)  # ACT-reduce mandatory out

        def coef_pair(edge_eo, adj_ap, accum, nob):
            # DVE: prod = edge_e * adj (bf16, 2x); ScalarE: accum = sum(prod)
            prod = prod_pool.tile([P, N], BF16, tag="prod")
            nc.vector.tensor_tensor(
                out=prod[:, :nob], in0=edge_eo, in1=adj_ap, op=MUL
            )
            nc.scalar.activation(
                out=scratch[:, :nob], in_=prod[:, :nob], func=COPY, accum_out=accum
            )

        for t in range(NT):
            # bf16 adj row for this tile
            nc.scalar.copy(adj_bf[:, t, :], adj_f32[t][:])

            coef = work_pool.tile([P, E], F32, tag="coef")
            if t in SPLIT_TILES:
                coefh = work_pool.tile([P, 2, E], F32, tag="coefh")
                for h in range(2):
                    eh = edge_tiles[t][h]
                    for e in range(E):
                        coef_pair(
                            eh[:, e, :],
                            adj_bf[:, t, bass.ts(h, H)],
                            coefh[:, h, e : e + 1],
                            H,
                        )
                nc.vector.tensor_tensor(
                    out=coef[:], in0=coefh[:, 0, :], in1=coefh[:, 1, :], op=ADD
                )
            else:
                et = edge_tiles[t]
                for e in range(E):
                    coef_pair(et[:, e, :], adj_bf[:, t, :], coef[:, e : e + 1], N)

            # V[s, e, i] for 4 e's per matmul (512-col moving operand), f32.
            psums = []
            for g in range(E // 4):
                pv = psum_pool.tile([P, 4, D], F32, tag="psum")
                nc.tensor.matmul(
                    pv[:],
                    lhsT=nodeT[:, bass.ts(t, P)],
                    rhs=wT[:, g * 4 : (g + 1) * 4, :],
                    start=True,
                    stop=True,
                )
                psums.append(pv)

            # out[s, i] = sum_e coef[s, e] * v[s, e, i]
            slab = slab_pool.tile([P, D, E], F32, tag="slab")
            for e in range(E):
                nc.scalar.mul(
                    slab[:, :, e], psums[e // 4][:, e % 4, :], coef[:, e : e + 1]
                )
            out_sb = work_pool.tile([P, D], F32, tag="out_sb")
            nc.vector.reduce_sum(out_sb[:], slab[:], axis=mybir.AxisListType.X)
            nc.sync.dma_start(out_d[bass.ts(t, P)], out_sb[:])

    nc.compile()
    return nc


_NC_CACHE = None


def get_nc():
    global _NC_CACHE
    if _NC_CACHE is None:
        _NC_CACHE = build_nc()
    return _NC_CACHE


def make_in_maps(node_state, edge_type_mat, adj_mat, W):
    # host-side: [B, N, N, E] -> [B, E, N, N] contiguous
    edge_t = np.ascontiguousarray(
        np.asarray(edge_type_mat, dtype=np.float32).transpose(0, 3, 1, 2)
    )
    return [
        {
            "node_state": np.ascontiguousarray(node_state[b], dtype=np.float32),
            "edge_type_mat": edge_t[b],
            "adj_mat": np.ascontiguousarray(adj_mat[b], dtype=np.float32),
            "W": np.ascontiguousarray(W, dtype=np.float32),
        }
        for b in range(B)
    ]


def kernel(node_state, edge_type_mat, adj_mat, W):
    nc = get_nc()
    in_maps = make_in_maps(node_state, edge_type_mat, adj_mat, W)
    res = run_bass_kernel_spmd(nc, in_maps, list(range(B)))
    return np.stack([res.results[b]["out"] for b in range(B)], axis=0)


# revision 8
# speedup vs baseline: 1.6107x; 1.0757x over previous
"""Trainium2 Bass kernel for nn_MessagePassing (gnn_message_passing).

Math (per batch b):
    coef[s,e] = sum_o adj[s,o] * edge[s,o,e]
    v[s,e,i]  = sum_j W[e,i,j] * node[s,j]
    out[s,i]  = sum_e coef[s,e] * v[s,e,i]

Sharding: data parallel over the batch axis — core b handles batch b.

Design notes (per core, ~36.5MB HBM read => ~103us DMA floor @358GB/s):
  * edge is transposed ON THE HOST to [E, N, N] so each e-stream is
    unit-stride on chip (strided SBUF reads cost ~2ns/elem on every
    engine — measured — which made the [s,o,e] layout uncompetitive).
  * edge is DMA'd with an f32->bf16 cast on the SWDGE (gpsimd) queue;
    all 8 s-tiles stay resident in SBUF (16KB/partition each) so the
    stream is never backpressured.  s-tiles 0/7 are split into o-halves
    to shrink pipeline ramp/tail.
  * adj: f32 per-tile chunks on the sync HWDGE queue (concurrent with
    the SWDGE stream), cast to bf16 by ScalarE on chip.
  * coef: per (s-tile, e) a DVE tensor_tensor bf16 multiply (2x_1p mode,
    ~0.69us) + a ScalarE activation-accumulate reduce (~0.4us).
    (scalar_tensor_tensor / tensor_tensor_reduce have no 2x uops.)
  * v: PE matmuls, f32 (nodeT stationary, W^T moving), independent of coef.
  * out: 8 ScalarE activation muls (scale = coef[:,e] per-partition) into
    a [P, i, e] slab, then one unit-stride DVE reduce over e.
"""

import numpy as np
from contextlib import ExitStack

import concourse.bass as bass
import concourse.bacc as bacc
import concourse.mybir as mybir
import concourse.tile as tile
from concourse.bass_utils import run_bass_kernel_spmd
from concourse.masks import make_identity

B, N, D, E = 8, 1024, 128, 8
P = 128
NT = N // P  # 8 s-tiles per core
H = N // 2

F32 = mybir.dt.float32
BF16 = mybir.dt.bfloat16
MUL = mybir.AluOpType.mult
ADD = mybir.AluOpType.add
COPY = mybir.ActivationFunctionType.Copy

SPLIT_TILES = (0, NT - 1)  # s-tiles processed in o-halves


def build_nc():
    nc = bacc.Bacc("TRN2", target_bir_lowering=False, debug=False, num_devices=B)

    node_d = nc.dram_tensor("node_state", [N, D], F32, kind="ExternalInput").ap()
    # transposed on the host: [E, N(s), N(o)]
    edge_d = nc.dram_tensor("edge_type_mat", [E, N, N], F32, kind="ExternalInput").ap()
    adj_d = nc.dram_tensor("adj_mat", [N, N], F32, kind="ExternalInput").ap()
    w_d = nc.dram_tensor("W", [E, D, D], F32, kind="ExternalInput").ap()
    out_d = nc.dram_tensor("out", [N, D], F32, kind="ExternalOutput").ap()

    with tile.TileContext(nc) as tc, ExitStack() as ctx:
        const_pool = ctx.enter_context(tc.tile_pool(name="const", bufs=1))
        edge_pool = ctx.enter_context(tc.tile_pool(name="edge", bufs=1))
        adjf_pool = ctx.enter_context(tc.tile_pool(name="adjf", bufs=2))
        work_pool = ctx.enter_context(tc.tile_pool(name="work", bufs=2))
        prod_pool = ctx.enter_context(tc.tile_pool(name="prod", bufs=3))
        slab_pool = ctx.enter_context(tc.tile_pool(name="slab", bufs=2))
        psum_pool = ctx.enter_context(tc.tile_pool(name="psum", bufs=8, space="PSUM"))

        ident = const_pool.tile([P, P], F32)
        make_identity(nc, ident[:])

        # --- edge stream: SWDGE cast f32 -> bf16, all tiles resident -------
        # dest [p, e, o]; src per partition-line: E blocks of N f32.
        edge_src = edge_d.rearrange("e (t p) o -> p t e o", p=P)
        edge_tiles = {}  # t -> ([P, E, N] bf16, or per-half [P, E, H])
        for t in range(NT):
            if t in SPLIT_TILES:
                halves = []
                for h in range(2):
                    eh = edge_pool.tile([P, E, H], BF16, tag=f"edgeh{t}{h}")
                    nc.gpsimd.dma_start(eh[:], edge_src[:, t, :, bass.ts(h, H)])
                    halves.append(eh)
                edge_tiles[t] = halves
            else:
                et = edge_pool.tile([P, E, N], BF16, tag=f"edge{t}")
                nc.gpsimd.dma_start(et[:], edge_src[:, t, :, :])
                edge_tiles[t] = et

        # --- adj: f32 chunks on sync HWDGE, cast to bf16 on ScalarE --------
        adj_bf = const_pool.tile([P, NT, N], BF16)
        adj_f32 = {}
        for t in range(NT):
            af = adjf_pool.tile([P, N], F32, tag="adjf")
            nc.sync.dma_start(af[:], adj_d[bass.ts(t, P)])
            adj_f32[t] = af

        node_all = const_pool.tile([P, NT, D], F32)
        nc.scalar.dma_start(node_all[:], node_d.rearrange("(t p) j -> p t j", p=P))
        w_all = const_pool.tile([P, E, D], F32)  # [i, e, j]
        nc.scalar.dma_start(w_all[:], w_d.rearrange("e i j -> i e j"))

        # node^T[j, s] and W[e]^T[j, i] via PE transpose (copies on DVE,
        # which is idle until the first edge tile lands).
        nodeT = const_pool.tile([P, N], F32)
        for t in range(NT):
            pt = psum_pool.tile([P, P], F32, tag="psum")
            nc.tensor.transpose(pt[:], node_all[:, t, :], ident[:])
            nc.vector.tensor_copy(nodeT[:, bass.ts(t, P)], pt[:])
        wT = const_pool.tile([P, E, D], F32)  # [j, e, i]
        for e in range(E):
            pt = psum_pool.tile([P, P], F32, tag="psum")
            nc.tensor.transpose(pt[:], w_all[:, e, :], ident[:])
            nc.vector.tensor_copy(wT[:, e, :], pt[:])

        scratch = const_pool.tile([P, N], BF16)   # ACT-reduce mandatory out
        scratch2 = const_pool.tile([P, N], BF16)  # TTR mandatory product out

        N_TTR = 0  # e's per tile reduced fully on DVE (fused TTR)

        def coef_one(edge_eo, adj_ap, accum, nob, e):
            if e < E - N_TTR:
                # DVE: prod = edge_e * adj (bf16, 2x); ScalarE: accum = sum
                prod = prod_pool.tile([P, N], BF16, tag="prod")
                nc.vector.tensor_tensor(
                    out=prod[:, :nob], in0=edge_eo, in1=adj_ap, op=MUL
                )
                nc.scalar.activation(
                    out=scratch[:, :nob], in_=prod[:, :nob], func=COPY,
                    accum_out=accum,
                )
            else:
                # fused product+reduce, all on DVE (1x)
                nc.vector.tensor_tensor_reduce(
                    out=scratch2[:, :nob], in0=edge_eo, in1=adj_ap,
                    scale=1.0, scalar=0.0, op0=MUL, op1=ADD, accum_out=accum,
                )

        def coef_tile(t):
            coef = work_pool.tile([P, E], F32, tag="coef")
            if t in SPLIT_TILES:
                coefh = work_pool.tile([P, 2, E], F32, tag="coefh")
                for h in range(2):
                    eh = edge_tiles[t][h]
                    for e in range(E):
                        coef_one(
                            eh[:, e, :], adj_bf[:, t, bass.ts(h, H)],
                            coefh[:, h, e : e + 1], H, e,
                        )
                nc.vector.tensor_tensor(
                    out=coef[:], in0=coefh[:, 0, :], in1=coefh[:, 1, :], op=ADD
                )
            else:
                et = edge_tiles[t]
                for e in range(E):
                    coef_one(et[:, e, :], adj_bf[:, t, :], coef[:, e : e + 1], N, e)
            return coef

        def v_tile(t):
            # V[s, e, i] for 4 e's per matmul (512-col moving operand), f32.
            psums = []
            for g in range(E // 4):
                pv = psum_pool.tile([P, 4, D], F32, tag="psum")
                nc.tensor.matmul(
                    pv[:],
                    lhsT=nodeT[:, bass.ts(t, P)],
                    rhs=wT[:, g * 4 : (g + 1) * 4, :],
                    start=True,
                    stop=True,
                )
                psums.append(pv)
            return psums

        def out_tile(t, coef, psums):
            # out[s,i] = sum_e coef[s,e] * v[s,e,i]: chained STT on DVE.
            acc_a = work_pool.tile([P, D], F32, tag="acc_a")
            acc_b = work_pool.tile([P, D], F32, tag="acc_b")
            nc.vector.tensor_scalar_mul(acc_a[:], psums[0][:, 0, :], coef[:, 0:1])
            cur, nxt = acc_a, acc_b
            for e in range(1, E):
                nc.vector.scalar_tensor_tensor(
                    out=nxt[:],
                    in0=psums[e // 4][:, e % 4, :],
                    scalar=coef[:, e : e + 1],
                    in1=cur[:],
                    op0=MUL,
                    op1=ADD,
                )
                cur, nxt = nxt, cur
            nc.sync.dma_start(out_d[bass.ts(t, P)], cur[:])

        # Software-pipelined: coef(t) runs on DVE/ScalarE while the previous
        # tile's output chain (which waits on cross-engine reduces) drains.
        pending = None  # (t, coef, psums)
        for t in range(NT):
            nc.scalar.copy(adj_bf[:, t, :], adj_f32[t][:])  # bf16 cast
            coef = coef_tile(t)
            psums = v_tile(t)
            if pending is not None:
                out_tile(*pending)
            pending = (t, coef, psums)
        out_tile(*pending)

    nc.compile()
    return nc


_NC_CACHE = None


def get_nc():
    global _NC_CACHE
    if _NC_CACHE is None:
        _NC_CACHE = build_nc()
    return _NC_CACHE


def make_in_maps(node_state, edge_type_mat, adj_mat, W):
    # host-side: [B, N, N, E] -> [B, E, N, N] contiguous
    edge_t = np.ascontiguousarray(
        np.asarray(edge_type_mat, dtype=np.float32).transpose(0, 3, 1, 2)
    )
    return [
        {
            "node_state": np.ascontiguousarray(node_state[b], dtype=np.float32),
            "edge_type_mat": edge_t[b],
            "adj_mat": np.ascontiguousarray(adj_mat[b], dtype=np.float32),
            "W": np.ascontiguousarray(W, dtype=np.float32),
        }
        for b in range(B)
    ]


def kernel(node_state, edge_type_mat, adj_mat, W):
    nc = get_nc()
    in_maps = make_in_maps(node_state, edge_type_mat, adj_mat, W)
    res = run_bass_kernel_spmd(nc, in_maps, list(range(B)))
    return np.stack([res.results[b]["out"] for b in range(B)], axis=0)
